# revision 2
# baseline (speedup 1.0000x reference)
"""Trainium2 Bass kernel for nn_ConvEnhanced (conv/attn/quantum fused head).

Reference math per sample (x is (16,) f32, all in [0,1)):
    cls  = sigmoid(dot(x, w) + b)
    attn = mean_j sigmoid(a * x_j)
    q    = mean_j sin^2(pi * x_j / 2)        (the threshold/where is a no-op for x >= 0)
    out  = alpha * cls * attn + (1 - alpha) * q

Device strategy (pure data parallel over 8 cores, 524288 samples/core):
  - x is cast to fp16 on the host (|rel err| <= 2^-11, far inside the 2e-2
    gate) and shipped as (128, 65536): partition p owns 4096 contiguous
    samples (16 contiguous fp16 each) -> pure-copy HWDGE DMA at half the
    f32 byte count.
  - ScalarE does the two transcendental passes (1 elem/cycle/lane, dtype
    independent -> this engine is the ~110us roofline of the kernel):
        th = tanh((a/2) * x)        [sigmoid(ax) = 0.5 + 0.5*tanh(ax/2)]
        cs = sin(pi/2 - pi*x)       [= cos(pi*x); sin^2(pi x/2) = (1-cos(pi x))/2]
    Tanh and Sin share one ACT table set (silu_and_others) -> single load.
  - TensorE does the per-sample segmented sums (16 elems along the free dim)
    as 16 PSUM-accumulating N=512 matmuls per reduction with stride-16 APs:
        S_wx  += diag(w_j) @ x[:, j::16]     (fp16 in, fp32 accum)
        S_th  += I @ th[:, j::16]
        S_cs  += I @ cs[:, j::16]
    fp16 weights get FWL, so LDWEIGHTS (~53ns) hides behind each 215ns
    matmul via the PE's background weight buffer.
  - ScalarE: t_c = tanh(0.5*S_wx + b/2)  ->  cls = 0.5*(1 + t_c)
  - VectorE tail combine:
        out = c0 + c1*t_c + c2*(S_th + t_c*S_th) + c3*S_cs
        c0 = alpha/4 + (1-alpha)/2, c1 = alpha/4, c2 = alpha/64, c3 = -(1-alpha)/32
"""

import numpy as np

try:
    import concourse.bass as bass  # noqa: F401
except ImportError:  # pragma: no cover
    import sys

    sys.path.insert(0, "/opt/trn_rl_repo")
    import concourse.bass as bass  # noqa: F401

B = 4_194_304  # total samples
N_CORES = 8
P = 128  # partitions
KE = 16  # elements per sample (4x4 patch)
B_LOC = B // N_CORES  # samples per core
SPP = B_LOC // P  # samples per partition (4096)

_NC_CACHE = {}


def _build(spp, t_tile):
    """Build the Bass/Tile program for one core (SPMD: identical on all cores).

    spp:    samples per partition held by this core
    t_tile: samples per partition processed per tile iteration
    """
    import concourse.bacc as bacc
    import concourse.bass as bass
    import concourse.tile as tile
    from concourse import mybir

    F32 = mybir.dt.float32
    F16 = mybir.dt.float16
    A = mybir.ActivationFunctionType
    Op = mybir.AluOpType

    assert spp % t_tile == 0
    n_tiles = spp // t_tile
    ft = KE * t_tile  # free elems per tile per partition

    nc = bacc.Bacc("TRN2", target_bir_lowering=False)
    x_d = nc.declare_dram_parameter("x", [P, spp * KE], F16, isOutput=False)
    wd_d = nc.declare_dram_parameter("wdiag", [P, KE * P], F16, isOutput=False)
    id_d = nc.declare_dram_parameter("ident", [P, P], F16, isOutput=False)
    c_d = nc.declare_dram_parameter("consts", [P, 8], F32, isOutput=False)
    o_d = nc.declare_dram_parameter("out", [P, spp], F32, isOutput=True)

    PI = float(np.pi)

    with tile.TileContext(nc) as tc:
        with (
            tc.tile_pool(name="const", bufs=1) as cpool,
            tc.tile_pool(name="xp", bufs=2) as xpool,
            tc.tile_pool(name="actp", bufs=2) as apool,
            tc.tile_pool(name="smallp", bufs=2) as spool,
            tc.tile_pool(name="psump", bufs=2, space="PSUM") as ppool,
        ):
            wd_sb = cpool.tile([P, KE * P], F16, tag="wd")
            nc.sync.dma_start(wd_sb[:], wd_d[:])
            id_sb = cpool.tile([P, P], F16, tag="id")
            nc.sync.dma_start(id_sb[:], id_d[:])
            c_sb = cpool.tile([P, 8], F32, tag="c")
            nc.sync.dma_start(c_sb[:], c_d[:])

            wd_v = wd_sb[:].rearrange("p (j m) -> p j m", j=KE)

            for t in range(n_tiles):
                x_t = xpool.tile([P, ft], F16, tag="x")
                nc.sync.dma_start(x_t[:], x_d[:, bass.ts(t, ft)])

                # th = tanh((a/2) x), cs = sin(pi/2 - pi x) = cos(pi x)
                th_t = apool.tile([P, ft], F16, tag="th")
                nc.scalar.activation(th_t[:], x_t[:], A.Tanh, scale=c_sb[:, 0:1])
                cs_t = apool.tile([P, ft], F16, tag="cs")
                nc.scalar.activation(
                    cs_t[:], x_t[:], A.Sin, bias=c_sb[:, 6:7], scale=-PI
                )

                ps_wx = ppool.tile([P, t_tile], F32, tag="pwx")
                ps_th = ppool.tile([P, t_tile], F32, tag="pth")
                ps_cs = ppool.tile([P, t_tile], F32, tag="pcs")

                x_v = x_t[:].rearrange("p (t j) -> p t j", j=KE)
                th_v = th_t[:].rearrange("p (t j) -> p t j", j=KE)
                cs_v = cs_t[:].rearrange("p (t j) -> p t j", j=KE)

                for j in range(KE):
                    nc.tensor.matmul(
                        ps_wx[:],
                        lhsT=wd_v[:, j, :],
                        rhs=x_v[:, :, j],
                        start=(j == 0),
                        stop=(j == KE - 1),
                    )
                for j in range(KE):
                    nc.tensor.matmul(
                        ps_th[:],
                        lhsT=id_sb[:],
                        rhs=th_v[:, :, j],
                        start=(j == 0),
                        stop=(j == KE - 1),
                    )
                for j in range(KE):
                    nc.tensor.matmul(
                        ps_cs[:],
                        lhsT=id_sb[:],
                        rhs=cs_v[:, :, j],
                        start=(j == 0),
                        stop=(j == KE - 1),
                    )

                # t_c = tanh(0.5*S_wx + b/2); cls = 0.5*(1+t_c)
                tc_t = spool.tile([P, t_tile], F32, tag="tc")
                nc.scalar.activation(
                    tc_t[:], ps_wx[:], A.Tanh, bias=c_sb[:, 1:2], scale=0.5
                )
                # m1 = t_c * S_th ; a1 = S_th + m1
                m1 = spool.tile([P, t_tile], F32, tag="m1")
                nc.vector.tensor_mul(m1[:], tc_t[:], ps_th[:])
                a1 = spool.tile([P, t_tile], F32, tag="a1")
                nc.vector.tensor_add(a1[:], m1[:], ps_th[:])
                # t1 = c1*t_c + c0
                t1 = spool.tile([P, t_tile], F32, tag="t1")
                nc.vector.tensor_scalar(
                    t1[:], tc_t[:], c_sb[:, 2:3], c_sb[:, 3:4], Op.mult, Op.add
                )
                # p1 = c3*S_cs + t1
                p1 = spool.tile([P, t_tile], F32, tag="p1")
                nc.vector.scalar_tensor_tensor(
                    p1[:], ps_cs[:], c_sb[:, 5:6], t1[:], Op.mult, Op.add
                )
                # out = c2*a1 + p1
                o_t = spool.tile([P, t_tile], F32, tag="o")
                nc.vector.scalar_tensor_tensor(
                    o_t[:], a1[:], c_sb[:, 4:5], p1[:], Op.mult, Op.add
                )
                nc.sync.dma_start(o_d[:, bass.ts(t, t_tile)], o_t[:])

    # Pin Tanh+Sin to the one table set that holds both (silu_and_others) so
    # the act-table pass emits a single load instead of flip-flopping between
    # trig_and_small and exp_and_others every tile (~2.7us per switch on the
    # bottleneck engine). Indices/order of the table dict are preserved, so
    # act_func_set_id stays consistent with act_info.json.
    import concourse.hw_specs as hw_specs

    _orig_gat = hw_specs.get_activation_tables

    def _pinned_tables(arch):
        tabs = {k: set(v) for k, v in _orig_gat(arch).items()}
        assert A.Tanh in tabs["silu_and_others"] and A.Sin in tabs["silu_and_others"]
        for name, fns in tabs.items():
            if name != "silu_and_others":
                fns.discard(A.Tanh)
                fns.discard(A.Sin)
        return tabs

    bacc.get_activation_tables = _pinned_tables
    try:
        nc.compile()
    finally:
        bacc.get_activation_tables = _orig_gat
    return nc


def get_nc(spp=SPP, t_tile=512):
    key = (spp, t_tile)
    if key not in _NC_CACHE:
        _NC_CACHE[key] = _build(spp, t_tile)
    return _NC_CACHE[key]


def make_const_inputs(conv_w, conv_b, attn_w, alpha):
    """Host-side packing of the tiny runtime parameters into device tensors."""
    w = np.asarray(conv_w, dtype=np.float32).reshape(KE)
    b = float(np.asarray(conv_b, dtype=np.float32).reshape(-1)[0])
    a = float(np.asarray(attn_w, dtype=np.float32).reshape(-1)[0])
    al = float(np.asarray(alpha, dtype=np.float32))

    wdiag = np.zeros((P, KE, P), dtype=np.float16)
    idx = np.arange(P)
    wdiag[idx, :, idx] = w[None, :].astype(np.float16)
    wdiag = np.ascontiguousarray(wdiag.reshape(P, KE * P))

    ident = np.ascontiguousarray(np.eye(P, dtype=np.float16))

    row = np.zeros(8, dtype=np.float32)
    row[0] = a / 2.0  # scale for tanh(a x / 2)
    row[1] = b / 2.0  # bias for tanh(0.5 S_wx + b/2)
    row[2] = al / 4.0  # c1
    row[3] = al / 4.0 + (1.0 - al) / 2.0  # c0
    row[4] = al / 64.0  # c2
    row[5] = -(1.0 - al) / 32.0  # c3
    row[6] = np.pi / 2.0  # bias for sin(pi/2 - pi x) = cos(pi x)
    consts = np.ascontiguousarray(np.tile(row[None, :], (P, 1)))
    return wdiag, ident, consts


def prep_x(x):
    """Cast the full f32 input to fp16 and shard it (cores, P, spp*KE)."""
    x = np.asarray(x)
    assert x.size == B * KE
    return np.ascontiguousarray(x.reshape(N_CORES, P, SPP * KE).astype(np.float16))


def kernel(x, conv_w, conv_b, attn_w, alpha):
    from concourse.bass_utils import run_bass_kernel_spmd

    xs = prep_x(x)
    wdiag, ident, consts = make_const_inputs(conv_w, conv_b, attn_w, alpha)

    nc = get_nc()
    in_maps = [
        {"x": xs[c], "wdiag": wdiag, "ident": ident, "consts": consts}
        for c in range(N_CORES)
    ]
    res = run_bass_kernel_spmd(nc, in_maps, list(range(N_CORES)))
    out = np.concatenate(
        [np.asarray(res.results[c]["out"], dtype=np.float32).reshape(-1) for c in range(N_CORES)]
    )
    return out


# revision 6
# speedup vs baseline: 2.5266x; 2.5266x over previous
"""Trainium2 Bass kernel for nn_ConvEnhanced (conv/attn/quantum fused head).

Reference math per sample (x is (16,) f32, all in [0,1)):
    cls  = sigmoid(dot(x, w) + b)
    attn = mean_j sigmoid(a * x_j)
    q    = mean_j sin^2(pi * x_j / 2)        (the threshold/where is a no-op for x >= 0)
    out  = alpha * cls * attn + (1 - alpha) * q

Device strategy (pure data parallel over 8 cores, 524288 samples/core):
  - x is cast to fp16 on the host (|rel err| <= 2^-11, far inside the 2e-2
    gate) and shipped as (128, 65536): partition p owns 4096 samples packed
    per device tile as [n_tiles, KE, t_tile] (element-major within a tile) so
    every matmul rhs below is a CONTIGUOUS 1KB run per partition -- a strided
    (j::16) rhs measures ~5x slower PE streaming. Pure-copy HWDGE DMA at half
    the f32 byte count.
  - ScalarE does the two transcendental passes (1 elem/cycle/lane, dtype
    independent -> this engine is the ~110us roofline of the kernel):
        th = tanh((a/2) * x)        [sigmoid(ax) = 0.5 + 0.5*tanh(ax/2)]
        cs = sin(pi/2 - pi*x)       [= cos(pi*x); sin^2(pi x/2) = (1-cos(pi x))/2]
    Tanh and Sin share one ACT table set (silu_and_others) -> single load.
  - TensorE does the per-sample segmented sums (16 elems along the free dim)
    as 16 PSUM-accumulating N=512 matmuls per reduction with stride-16 APs:
        S_wx  += diag(w_j) @ x[:, j::16]     (fp16 in, fp32 accum)
        S_th  += I @ th[:, j::16]
        S_cs  += I @ cs[:, j::16]
    fp16 weights get FWL, so LDWEIGHTS (~53ns) hides behind each 215ns
    matmul via the PE's background weight buffer.
  - ScalarE: t_c = tanh(0.5*S_wx + b/2)  ->  cls = 0.5*(1 + t_c)
  - VectorE tail combine:
        out = c0 + c1*t_c + c2*(S_th + t_c*S_th) + c3*S_cs
        c0 = alpha/4 + (1-alpha)/2, c1 = alpha/4, c2 = alpha/64, c3 = -(1-alpha)/32
"""

import numpy as np

try:
    import concourse.bass as bass  # noqa: F401
except ImportError:  # pragma: no cover
    import sys

    sys.path.insert(0, "/opt/trn_rl_repo")
    import concourse.bass as bass  # noqa: F401

B = 4_194_304  # total samples
N_CORES = 8
P = 128  # partitions
KE = 16  # elements per sample (4x4 patch)
B_LOC = B // N_CORES  # samples per core
SPP = B_LOC // P  # samples per partition (4096)

_NC_CACHE = {}


def _build(spp, t_tile):
    """Build the Bass/Tile program for one core (SPMD: identical on all cores).

    spp:    samples per partition held by this core
    t_tile: samples per partition processed per tile iteration
    """
    import concourse.bacc as bacc
    import concourse.bass as bass
    import concourse.tile as tile
    from concourse import mybir

    F32 = mybir.dt.float32
    F16 = mybir.dt.float16
    A = mybir.ActivationFunctionType
    Op = mybir.AluOpType

    assert spp % t_tile == 0
    n_tiles = spp // t_tile
    ft = KE * t_tile  # free elems per tile per partition

    nc = bacc.Bacc("TRN2", target_bir_lowering=False)
    x_d = nc.declare_dram_parameter("x", [P, spp * KE], F16, isOutput=False)
    wd_d = nc.declare_dram_parameter("wdiag", [P, KE * P], F16, isOutput=False)
    id_d = nc.declare_dram_parameter("ident", [P, P], F16, isOutput=False)
    c_d = nc.declare_dram_parameter("consts", [P, 8], F32, isOutput=False)
    o_d = nc.declare_dram_parameter("out", [P, spp], F32, isOutput=True)

    PI = float(np.pi)

    with tile.TileContext(nc) as tc:
        with (
            tc.tile_pool(name="const", bufs=1) as cpool,
            tc.tile_pool(name="xp", bufs=2) as xpool,
            tc.tile_pool(name="actp", bufs=2) as apool,
            tc.tile_pool(name="smallp", bufs=2) as spool,
            tc.tile_pool(name="psump", bufs=2, space="PSUM") as ppool,
        ):
            wd_sb = cpool.tile([P, KE * P], F16, tag="wd")
            nc.sync.dma_start(wd_sb[:], wd_d[:])
            id_sb = cpool.tile([P, P], F16, tag="id")
            nc.sync.dma_start(id_sb[:], id_d[:])
            c_sb = cpool.tile([P, 8], F32, tag="c")
            nc.sync.dma_start(c_sb[:], c_d[:])

            wd_v = wd_sb[:].rearrange("p (j m) -> p j m", j=KE)

            for t in range(n_tiles):
                x_t = xpool.tile([P, ft], F16, tag="x")
                nc.sync.dma_start(x_t[:], x_d[:, bass.ts(t, ft)])

                # th = tanh((a/2) x), cs = sin(pi/2 - pi x) = cos(pi x)
                th_t = apool.tile([P, ft], F16, tag="th")
                nc.scalar.activation(th_t[:], x_t[:], A.Tanh, scale=c_sb[:, 0:1])
                cs_t = apool.tile([P, ft], F16, tag="cs")
                nc.scalar.activation(
                    cs_t[:], x_t[:], A.Sin, bias=c_sb[:, 6:7], scale=-PI
                )

                ps_wx = ppool.tile([P, t_tile], F32, tag="pwx")
                ps_th = ppool.tile([P, t_tile], F32, tag="pth")
                ps_cs = ppool.tile([P, t_tile], F32, tag="pcs")

                # tile-packed layout: columns [j*t_tile, (j+1)*t_tile) hold
                # element j of every sample in the tile -> contiguous rhs
                for j in range(KE):
                    nc.tensor.matmul(
                        ps_wx[:],
                        lhsT=wd_v[:, j, :],
                        rhs=x_t[:, bass.ts(j, t_tile)],
                        start=(j == 0),
                        stop=(j == KE - 1),
                    )
                for j in range(KE):
                    nc.tensor.matmul(
                        ps_th[:],
                        lhsT=id_sb[:],
                        rhs=th_t[:, bass.ts(j, t_tile)],
                        start=(j == 0),
                        stop=(j == KE - 1),
                    )
                for j in range(KE):
                    nc.tensor.matmul(
                        ps_cs[:],
                        lhsT=id_sb[:],
                        rhs=cs_t[:, bass.ts(j, t_tile)],
                        start=(j == 0),
                        stop=(j == KE - 1),
                    )

                # t_c = tanh(0.5*S_wx + b/2); cls = 0.5*(1+t_c)
                tc_t = spool.tile([P, t_tile], F32, tag="tc")
                nc.scalar.activation(
                    tc_t[:], ps_wx[:], A.Tanh, bias=c_sb[:, 1:2], scale=0.5
                )
                # m1 = t_c * S_th ; a1 = S_th + m1
                m1 = spool.tile([P, t_tile], F32, tag="m1")
                nc.vector.tensor_mul(m1[:], tc_t[:], ps_th[:])
                a1 = spool.tile([P, t_tile], F32, tag="a1")
                nc.vector.tensor_add(a1[:], m1[:], ps_th[:])
                # t1 = c1*t_c + c0
                t1 = spool.tile([P, t_tile], F32, tag="t1")
                nc.vector.tensor_scalar(
                    t1[:], tc_t[:], c_sb[:, 2:3], c_sb[:, 3:4], Op.mult, Op.add
                )
                # p1 = c3*S_cs + t1
                p1 = spool.tile([P, t_tile], F32, tag="p1")
                nc.vector.scalar_tensor_tensor(
                    p1[:], ps_cs[:], c_sb[:, 5:6], t1[:], Op.mult, Op.add
                )
                # out = c2*a1 + p1
                o_t = spool.tile([P, t_tile], F32, tag="o")
                nc.vector.scalar_tensor_tensor(
                    o_t[:], a1[:], c_sb[:, 4:5], p1[:], Op.mult, Op.add
                )
                nc.sync.dma_start(o_d[:, bass.ts(t, t_tile)], o_t[:])

    # Pin Tanh+Sin to the one table set that holds both (silu_and_others) so
    # the act-table pass emits a single load instead of flip-flopping between
    # trig_and_small and exp_and_others every tile (~2.7us per switch on the
    # bottleneck engine). Indices/order of the table dict are preserved, so
    # act_func_set_id stays consistent with act_info.json.
    import concourse.hw_specs as hw_specs

    _orig_gat = hw_specs.get_activation_tables

    def _pinned_tables(arch):
        tabs = {k: set(v) for k, v in _orig_gat(arch).items()}
        assert A.Tanh in tabs["silu_and_others"] and A.Sin in tabs["silu_and_others"]
        for name, fns in tabs.items():
            if name != "silu_and_others":
                fns.discard(A.Tanh)
                fns.discard(A.Sin)
        return tabs

    bacc.get_activation_tables = _pinned_tables
    try:
        nc.compile()
    finally:
        bacc.get_activation_tables = _orig_gat
    return nc


def get_nc(spp=SPP, t_tile=None):
    if t_tile is None:
        t_tile = T_TILE
    key = (spp, t_tile)
    if key not in _NC_CACHE:
        _NC_CACHE[key] = _build(spp, t_tile)
    return _NC_CACHE[key]


def make_const_inputs(conv_w, conv_b, attn_w, alpha):
    """Host-side packing of the tiny runtime parameters into device tensors."""
    w = np.asarray(conv_w, dtype=np.float32).reshape(KE)
    b = float(np.asarray(conv_b, dtype=np.float32).reshape(-1)[0])
    a = float(np.asarray(attn_w, dtype=np.float32).reshape(-1)[0])
    al = float(np.asarray(alpha, dtype=np.float32))

    wdiag = np.zeros((P, KE, P), dtype=np.float16)
    idx = np.arange(P)
    wdiag[idx, :, idx] = w[None, :].astype(np.float16)
    wdiag = np.ascontiguousarray(wdiag.reshape(P, KE * P))

    ident = np.ascontiguousarray(np.eye(P, dtype=np.float16))

    row = np.zeros(8, dtype=np.float32)
    row[0] = a / 2.0  # scale for tanh(a x / 2)
    row[1] = b / 2.0  # bias for tanh(0.5 S_wx + b/2)
    row[2] = al / 4.0  # c1
    row[3] = al / 4.0 + (1.0 - al) / 2.0  # c0
    row[4] = al / 64.0  # c2
    row[5] = -(1.0 - al) / 32.0  # c3
    row[6] = np.pi / 2.0  # bias for sin(pi/2 - pi x) = cos(pi x)
    consts = np.ascontiguousarray(np.tile(row[None, :], (P, 1)))
    return wdiag, ident, consts


T_TILE = 512  # samples per partition per device tile (PSUM bank = 512 f32)


def pack_x(x3d, t_tile):
    """[..., spp, KE] f32 -> [..., spp*KE] fp16, tile-packed element-major.

    Within each device tile of t_tile samples, all KE=16 element-0 values come
    first, then element-1, ... so each matmul rhs is a contiguous run.
    """
    *lead, spp, ke = x3d.shape
    n_tiles = spp // t_tile
    v = x3d.astype(np.float16).reshape(*lead, n_tiles, t_tile, ke)
    v = np.swapaxes(v, -1, -2)
    return np.ascontiguousarray(v.reshape(*lead, spp * ke))


def prep_x(x, t_tile=T_TILE):
    """Cast the full f32 input to fp16, shard and tile-pack (cores, P, spp*KE)."""
    x = np.asarray(x)
    assert x.size == B * KE
    return pack_x(x.reshape(N_CORES, P, SPP, KE), t_tile)


def kernel(x, conv_w, conv_b, attn_w, alpha):
    from concourse.bass_utils import run_bass_kernel_spmd

    xs = prep_x(x)
    wdiag, ident, consts = make_const_inputs(conv_w, conv_b, attn_w, alpha)

    nc = get_nc()
    in_maps = [
        {"x": xs[c], "wdiag": wdiag, "ident": ident, "consts": consts}
        for c in range(N_CORES)
    ]
    res = run_bass_kernel_spmd(nc, in_maps, list(range(N_CORES)))
    out = np.concatenate(
        [np.asarray(res.results[c]["out"], dtype=np.float32).reshape(-1) for c in range(N_CORES)]
    )
    return out


# revision 10
# speedup vs baseline: 2.5896x; 1.0249x over previous
"""Trainium2 Bass kernel for nn_ConvEnhanced (conv/attn/quantum fused head).

Reference math per sample (x is (16,) f32, all in [0,1)):
    cls  = sigmoid(dot(x, w) + b)
    attn = mean_j sigmoid(a * x_j)
    q    = mean_j sin^2(pi * x_j / 2)        (the threshold/where is a no-op for x >= 0)
    out  = alpha * cls * attn + (1 - alpha) * q

Device strategy (pure data parallel over 8 cores, 524288 samples/core):
  - x is cast to fp16 on the host (|rel err| <= 2^-11, far inside the 2e-2
    gate) and shipped as (128, 65536): partition p owns 4096 samples packed
    per device tile as [n_tiles, KE, t_tile] (element-major within a tile) so
    every matmul rhs below is a CONTIGUOUS 1KB run per partition -- a strided
    (j::16) rhs measures ~5x slower PE streaming. Pure-copy HWDGE DMA at half
    the f32 byte count.
  - ScalarE does the two transcendental passes (1 elem/cycle/lane, dtype
    independent -> this engine is the ~110us roofline of the kernel):
        th = tanh((a/2) * x)        [sigmoid(ax) = 0.5 + 0.5*tanh(ax/2)]
        cs = sin(pi/2 - pi*x)       [= cos(pi*x); sin^2(pi x/2) = (1-cos(pi x))/2]
    Tanh and Sin share one ACT table set (silu_and_others) -> single load.
  - TensorE does the per-sample segmented sums (16 elems along the free dim)
    as 16 PSUM-accumulating N=512 matmuls per reduction with stride-16 APs:
        S_wx  += diag(w_j) @ x[:, j::16]     (fp16 in, fp32 accum)
        S_th  += I @ th[:, j::16]
        S_cs  += I @ cs[:, j::16]
    fp16 weights get FWL, so LDWEIGHTS (~53ns) hides behind each 215ns
    matmul via the PE's background weight buffer.
  - ScalarE: t_c = tanh(0.5*S_wx + b/2)  ->  cls = 0.5*(1 + t_c)
  - VectorE tail combine:
        out = c0 + c1*t_c + c2*(S_th + t_c*S_th) + c3*S_cs
        c0 = alpha/4 + (1-alpha)/2, c1 = alpha/4, c2 = alpha/64, c3 = -(1-alpha)/32
"""

import numpy as np

try:
    import concourse.bass as bass  # noqa: F401
except ImportError:  # pragma: no cover
    import sys

    sys.path.insert(0, "/opt/trn_rl_repo")
    import concourse.bass as bass  # noqa: F401

B = 4_194_304  # total samples
N_CORES = 8
P = 128  # partitions
KE = 16  # elements per sample (4x4 patch)
B_LOC = B // N_CORES  # samples per core
SPP = B_LOC // P  # samples per partition (4096)

_NC_CACHE = {}

# Per-device-tile sample counts (per partition). Small edge tiles shrink the
# pipeline fill (first x DMA) and drain (last tile's MM+tail chain); 512-deep
# middle tiles amortize ACT/MM instruction overhead. Sum must equal SPP.
TILES = (256, 512, 512, 512, 512, 512, 512, 512, 128, 128)


def _build(spp, tiles):
    """Build the Bass/Tile program for one core (SPMD: identical on all cores).

    spp:   samples per partition held by this core
    tiles: per-iteration sample counts (per partition), summing to spp
    """
    import concourse.bacc as bacc
    import concourse.bass as bass
    import concourse.tile as tile
    from concourse import mybir

    F32 = mybir.dt.float32
    F16 = mybir.dt.float16
    A = mybir.ActivationFunctionType
    Op = mybir.AluOpType

    tiles = list(tiles)
    assert sum(tiles) == spp
    t_max = max(tiles)
    ft_max = KE * t_max

    nc = bacc.Bacc("TRN2", target_bir_lowering=False)
    x_d = nc.declare_dram_parameter("x", [P, spp * KE], F16, isOutput=False)
    wd_d = nc.declare_dram_parameter("wdiag", [P, KE * P], F16, isOutput=False)
    id_d = nc.declare_dram_parameter("ident", [P, P], F16, isOutput=False)
    c_d = nc.declare_dram_parameter("consts", [P, 8], F32, isOutput=False)
    o_d = nc.declare_dram_parameter("out", [P, spp], F32, isOutput=True)

    PI = float(np.pi)

    with tile.TileContext(nc) as tc:
        with (
            tc.tile_pool(name="const", bufs=1) as cpool,
            tc.tile_pool(name="xp", bufs=2) as xpool,
            tc.tile_pool(name="actp", bufs=2) as apool,
            tc.tile_pool(name="smallp", bufs=2) as spool,
            tc.tile_pool(name="psump", bufs=2, space="PSUM") as ppool,
        ):
            # Consts go on the gpsimd (SWDGE) queue so the sync queue's first
            # issue is the tile-0 x DMA (shortest path to the first ACT).
            wd_sb = cpool.tile([P, KE * P], F16, tag="wd")
            nc.gpsimd.dma_start(out=wd_sb[:], in_=wd_d[:])
            id_sb = cpool.tile([P, P], F16, tag="id")
            nc.gpsimd.dma_start(out=id_sb[:], in_=id_d[:])
            c_sb = cpool.tile([P, 8], F32, tag="c")
            nc.gpsimd.dma_start(out=c_sb[:], in_=c_d[:])

            # Dummy 1-element ACT: forces the (single, pinned) ACT table set
            # to load while the tile-0 x DMA is still in flight, instead of
            # serializing ~1.3us of table load after it lands.
            warm_i = cpool.tile([P, 1], F32, tag="warm_i")
            nc.gpsimd.memset(warm_i[:], 0.0)
            warm_o = cpool.tile([P, 1], F32, tag="warm_o")
            nc.scalar.activation(warm_o[:], warm_i[:], A.Tanh)

            wd_v = wd_sb[:].rearrange("p (j m) -> p j m", j=KE)

            off = 0
            for t_tile in tiles:
                ft = KE * t_tile
                e0 = off * KE  # element offset of this tile in DRAM
                x_t = xpool.tile([P, ft_max], F16, tag="x")
                nc.sync.dma_start(x_t[:, 0:ft], x_d[:, e0 : e0 + ft])

                # th = tanh((a/2) x), cs = sin(pi/2 - pi x) = cos(pi x)
                th_t = apool.tile([P, ft_max], F16, tag="th")
                nc.scalar.activation(
                    th_t[:, 0:ft], x_t[:, 0:ft], A.Tanh, scale=c_sb[:, 0:1]
                )
                cs_t = apool.tile([P, ft_max], F16, tag="cs")
                nc.scalar.activation(
                    cs_t[:, 0:ft], x_t[:, 0:ft], A.Sin, bias=c_sb[:, 6:7], scale=-PI
                )

                ps_wx = ppool.tile([P, t_max], F32, tag="pwx")
                ps_th = ppool.tile([P, t_max], F32, tag="pth")
                ps_cs = ppool.tile([P, t_max], F32, tag="pcs")

                # tile-packed layout: columns [j*t_tile, (j+1)*t_tile) hold
                # element j of every sample in the tile -> contiguous rhs
                for j in range(KE):
                    nc.tensor.matmul(
                        ps_wx[:, 0:t_tile],
                        lhsT=wd_v[:, j, :],
                        rhs=x_t[:, bass.ts(j, t_tile)],
                        start=(j == 0),
                        stop=(j == KE - 1),
                    )
                for j in range(KE):
                    nc.tensor.matmul(
                        ps_th[:, 0:t_tile],
                        lhsT=id_sb[:],
                        rhs=th_t[:, bass.ts(j, t_tile)],
                        start=(j == 0),
                        stop=(j == KE - 1),
                    )
                for j in range(KE):
                    nc.tensor.matmul(
                        ps_cs[:, 0:t_tile],
                        lhsT=id_sb[:],
                        rhs=cs_t[:, bass.ts(j, t_tile)],
                        start=(j == 0),
                        stop=(j == KE - 1),
                    )

                # t_c = tanh(0.5*S_wx + b/2); cls = 0.5*(1+t_c)
                tc_t = spool.tile([P, t_max], F32, tag="tc")
                nc.scalar.activation(
                    tc_t[:, 0:t_tile], ps_wx[:, 0:t_tile], A.Tanh,
                    bias=c_sb[:, 1:2], scale=0.5,
                )
                # m1 = t_c * S_th ; a1 = S_th + m1
                m1 = spool.tile([P, t_max], F32, tag="m1")
                nc.vector.tensor_mul(m1[:, 0:t_tile], tc_t[:, 0:t_tile], ps_th[:, 0:t_tile])
                a1 = spool.tile([P, t_max], F32, tag="a1")
                nc.vector.tensor_add(a1[:, 0:t_tile], m1[:, 0:t_tile], ps_th[:, 0:t_tile])
                # t1 = c1*t_c + c0
                t1 = spool.tile([P, t_max], F32, tag="t1")
                nc.vector.tensor_scalar(
                    t1[:, 0:t_tile], tc_t[:, 0:t_tile],
                    c_sb[:, 2:3], c_sb[:, 3:4], Op.mult, Op.add,
                )
                # p1 = c3*S_cs + t1
                p1 = spool.tile([P, t_max], F32, tag="p1")
                nc.vector.scalar_tensor_tensor(
                    p1[:, 0:t_tile], ps_cs[:, 0:t_tile], c_sb[:, 5:6],
                    t1[:, 0:t_tile], Op.mult, Op.add,
                )
                # out = c2*a1 + p1
                o_t = spool.tile([P, t_max], F32, tag="o")
                nc.vector.scalar_tensor_tensor(
                    o_t[:, 0:t_tile], a1[:, 0:t_tile], c_sb[:, 4:5],
                    p1[:, 0:t_tile], Op.mult, Op.add,
                )
                nc.sync.dma_start(o_d[:, off : off + t_tile], o_t[:, 0:t_tile])
                off += t_tile

    # Pin Tanh+Sin to the one table set that holds both (silu_and_others) so
    # the act-table pass emits a single load instead of flip-flopping between
    # trig_and_small and exp_and_others every tile (~2.7us per switch on the
    # bottleneck engine). Indices/order of the table dict are preserved, so
    # act_func_set_id stays consistent with act_info.json.
    import concourse.hw_specs as hw_specs

    _orig_gat = hw_specs.get_activation_tables

    def _pinned_tables(arch):
        tabs = {k: set(v) for k, v in _orig_gat(arch).items()}
        assert A.Tanh in tabs["silu_and_others"] and A.Sin in tabs["silu_and_others"]
        for name, fns in tabs.items():
            if name != "silu_and_others":
                fns.discard(A.Tanh)
                fns.discard(A.Sin)
        return tabs

    bacc.get_activation_tables = _pinned_tables
    try:
        nc.compile()
    finally:
        bacc.get_activation_tables = _orig_gat
    return nc


def get_nc(spp=SPP, tiles=None):
    if tiles is None:
        tiles = TILES
    key = (spp, tuple(tiles))
    if key not in _NC_CACHE:
        _NC_CACHE[key] = _build(spp, tiles)
    return _NC_CACHE[key]


def make_const_inputs(conv_w, conv_b, attn_w, alpha):
    """Host-side packing of the tiny runtime parameters into device tensors."""
    w = np.asarray(conv_w, dtype=np.float32).reshape(KE)
    b = float(np.asarray(conv_b, dtype=np.float32).reshape(-1)[0])
    a = float(np.asarray(attn_w, dtype=np.float32).reshape(-1)[0])
    al = float(np.asarray(alpha, dtype=np.float32))

    wdiag = np.zeros((P, KE, P), dtype=np.float16)
    idx = np.arange(P)
    wdiag[idx, :, idx] = w[None, :].astype(np.float16)
    wdiag = np.ascontiguousarray(wdiag.reshape(P, KE * P))

    ident = np.ascontiguousarray(np.eye(P, dtype=np.float16))

    row = np.zeros(8, dtype=np.float32)
    row[0] = a / 2.0  # scale for tanh(a x / 2)
    row[1] = b / 2.0  # bias for tanh(0.5 S_wx + b/2)
    row[2] = al / 4.0  # c1
    row[3] = al / 4.0 + (1.0 - al) / 2.0  # c0
    row[4] = al / 64.0  # c2
    row[5] = -(1.0 - al) / 32.0  # c3
    row[6] = np.pi / 2.0  # bias for sin(pi/2 - pi x) = cos(pi x)
    consts = np.ascontiguousarray(np.tile(row[None, :], (P, 1)))
    return wdiag, ident, consts


def pack_x(x3d, tiles):
    """[..., spp, KE] f32 -> [..., spp*KE] fp16, tile-packed element-major.

    Within each device tile of t samples, all t element-0 values come first,
    then element-1, ... so each matmul rhs is a contiguous run.
    """
    *lead, spp, ke = x3d.shape
    assert sum(tiles) == spp
    v = x3d.astype(np.float16)
    out = np.empty((*lead, spp * ke), dtype=np.float16)
    off = 0
    for t in tiles:
        seg = np.swapaxes(v[..., off : off + t, :], -1, -2)
        out[..., off * ke : (off + t) * ke] = seg.reshape(*lead, t * ke)
        off += t
    return out


def prep_x(x, tiles=TILES):
    """Cast the full f32 input to fp16, shard and tile-pack (cores, P, spp*KE)."""
    x = np.asarray(x)
    assert x.size == B * KE
    return pack_x(x.reshape(N_CORES, P, SPP, KE), tiles)


def kernel(x, conv_w, conv_b, attn_w, alpha):
    from concourse.bass_utils import run_bass_kernel_spmd

    xs = prep_x(x)
    wdiag, ident, consts = make_const_inputs(conv_w, conv_b, attn_w, alpha)

    nc = get_nc()
    in_maps = [
        {"x": xs[c], "wdiag": wdiag, "ident": ident, "consts": consts}
        for c in range(N_CORES)
    ]
    res = run_bass_kernel_spmd(nc, in_maps, list(range(N_CORES)))
    out = np.concatenate(
        [np.asarray(res.results[c]["out"], dtype=np.float32).reshape(-1) for c in range(N_CORES)]
    )
    return out
